# revision 2
# baseline (speedup 1.0000x reference)
"""Binarized 4-layer MLP (8192x784 -> 6144 -> 6144 -> 6144 -> 10, log_softmax)
on 8 Trainium2 NeuronCores, data-parallel over the batch.

Per-core dataflow (batch slice of 1024, feature-major activations [feat, batch]):
  fc1: x @ sign(w1).T   as 3-pass bf16 hi/mid/lo split of x (fp32-accurate,
       weights are exactly +-1 in bf16), accumulated in fp32 PSUM.
  fc2/fc3: sign(h) @ sign(w).T in fp8e4 with DoubleRow perf mode. All products
       are +-1 and partial sums are small integers, so fp32 PSUM accumulation
       is bit-exact regardless of order.
  fc4: fused into the fc3 m-loop; 2-pass bf16 (w4 split hi/lo), h3 in bf16.
  log_softmax: PE-transpose of the [10, 1024] logits to [1024, 10] tiles,
       then max/exp/sum/ln on Vector+Scalar engines.
"""

import numpy as np
import ml_dtypes

import concourse.bass as bass
import concourse.mybir as mybir
from concourse import bacc
from concourse.tile import TileContext
from concourse.bass_utils import run_bass_kernel_spmd
from concourse.masks import make_identity

dt = mybir.dt

CORES = 8
B = 8192
BC = B // CORES          # 1024 batch rows per core
DIN = 784
KT1 = 7                  # fc1 contraction tiles (784 padded to 896 = 7*128)
DINP = KT1 * 128
DH = 6144
MT = DH // 128           # 48 feature tiles
KB = DH // 256           # 24 DoubleRow contraction blocks
DOUT = 10
NH = BC // 512           # 2 moving halves of 512
MQ = 4                   # fc1 m-quarters (w1 streamed per quarter)

BF16 = ml_dtypes.bfloat16
FP8 = mybir.dt.np(dt.float8e4)

last_exec_time_ns = None


def _build_program(trace=False):
    nc = bacc.Bacc("TRN2", target_bir_lowering=False, debug=False,
                   num_devices=CORES)

    xt = nc.dram_tensor("xt", [128, KT1, 3, BC], dt.bfloat16,
                        kind="ExternalInput").ap()
    w1t = nc.dram_tensor("w1t", [KT1, 128, DH], dt.bfloat16,
                         kind="ExternalInput").ap()
    w2p = nc.dram_tensor("w2p", [MT, 128, KB, 2, 128], dt.float8e4,
                         kind="ExternalInput").ap()
    w3p = nc.dram_tensor("w3p", [MT, 128, KB, 2, 128], dt.float8e4,
                         kind="ExternalInput").ap()
    w4p = nc.dram_tensor("w4p", [128, 2, MT, DOUT], dt.bfloat16,
                         kind="ExternalInput").ap()
    b1p = nc.dram_tensor("b1p", [128, MT], dt.float32, kind="ExternalInput").ap()
    b2p = nc.dram_tensor("b2p", [128, MT], dt.float32, kind="ExternalInput").ap()
    b3p = nc.dram_tensor("b3p", [128, MT], dt.float32, kind="ExternalInput").ap()
    b4p = nc.dram_tensor("b4p", [DOUT, 1], dt.float32, kind="ExternalInput").ap()
    out = nc.dram_tensor("out", [BC, DOUT], dt.float32, kind="ExternalOutput").ap()

    DR = mybir.MatmulPerfMode.DoubleRow

    with TileContext(nc) as tc:
        with tc.tile_pool(name="consts", bufs=1) as cpool, \
             tc.tile_pool(name="h1p", bufs=1) as h1pool, \
             tc.tile_pool(name="h2p", bufs=1) as h2pool:
            xt_sb = cpool.tile([128, KT1, 3, BC], dt.bfloat16)
            nc.sync.dma_start(out=xt_sb[:], in_=xt[:])
            b1_sb = cpool.tile([128, MT], dt.float32)
            nc.sync.dma_start(out=b1_sb[:], in_=b1p[:])
            b2_sb = cpool.tile([128, MT], dt.float32)
            nc.sync.dma_start(out=b2_sb[:], in_=b2p[:])
            b3_sb = cpool.tile([128, MT], dt.float32)
            nc.sync.dma_start(out=b3_sb[:], in_=b3p[:])
            b4_sb = cpool.tile([DOUT, 1], dt.float32)
            nc.sync.dma_start(out=b4_sb[:], in_=b4p[:])
            w4_sb = cpool.tile([128, 2, MT, DOUT], dt.bfloat16)
            nc.sync.dma_start(out=w4_sb[:], in_=w4p[:])
            ident = cpool.tile([DOUT, DOUT], dt.float32)
            make_identity(nc, ident[:])

            h1 = h1pool.tile([128, MT, BC], dt.float8e4)
            h2 = h2pool.tile([128, MT, BC], dt.float8e4)

            # ---------------- fc1 ----------------
            MPQ = MT // MQ  # m-tiles per quarter
            with tc.tile_pool(name="w1pool", bufs=2) as w1pool, \
                 tc.tile_pool(name="ps1", bufs=3, space="PSUM") as ps1:
                for q in range(MQ):
                    w1q = w1pool.tile([128, KT1, MPQ * 128], dt.bfloat16, tag="w1q")
                    for k in range(KT1):
                        nc.sync.dma_start(
                            out=w1q[:, k, :],
                            in_=w1t[k, :, q * MPQ * 128:(q + 1) * MPQ * 128])
                    for mi in range(MPQ):
                        m = q * MPQ + mi
                        psum = ps1.tile([128, BC], dt.float32, tag="ps1")
                        for k in range(KT1):
                            lhsT = w1q[:, k, mi * 128:(mi + 1) * 128]
                            for s in range(3):
                                for n in range(NH):
                                    nc.tensor.matmul(
                                        psum[:, n * 512:(n + 1) * 512],
                                        lhsT,
                                        xt_sb[:, k, s, n * 512:(n + 1) * 512],
                                        start=(k == 0 and s == 0),
                                        stop=(k == KT1 - 1 and s == 2),
                                    )
                        nc.scalar.sign(h1[:, m, :], psum[:, :],
                                       bias=b1_sb[:, m:m + 1])

            # ---------------- fc2 ----------------
            with tc.tile_pool(name="w2pool", bufs=3) as w2pool, \
                 tc.tile_pool(name="ps2", bufs=3, space="PSUM") as ps2:
                for m in range(MT):
                    wsb = w2pool.tile([128, KB, 2, 128], dt.float8e4, tag="w2")
                    nc.sync.dma_start(out=wsb[:], in_=w2p[m])
                    psum = ps2.tile([128, BC], dt.float32, tag="ps2")
                    for b in range(KB):
                        for n in range(NH):
                            nc.tensor.matmul(
                                psum[:, n * 512:(n + 1) * 512],
                                wsb[:, b],
                                h1[:, 2 * b:2 * b + 2, n * 512:(n + 1) * 512],
                                start=(b == 0),
                                stop=(b == KB - 1),
                                perf_mode=DR,
                            )
                    nc.scalar.sign(h2[:, m, :], psum[:, :],
                                   bias=b2_sb[:, m:m + 1])

            # ---------------- fc3 + fused fc4 ----------------
            with tc.tile_pool(name="lgp", bufs=1, space="PSUM") as lgp, \
                 tc.tile_pool(name="lgsbp", bufs=1) as lgsbp:
                lg_psum = lgp.tile([DOUT, BC], dt.float32)
                with tc.tile_pool(name="w3pool", bufs=3) as w3pool, \
                     tc.tile_pool(name="h3pool", bufs=3) as h3pool, \
                     tc.tile_pool(name="ps3", bufs=3, space="PSUM") as ps3:
                    h3_tiles = [None] * MT

                    def fc4_mms(m):
                        t_bf = h3_tiles[m]
                        for p in range(2):
                            for n in range(NH):
                                nc.tensor.matmul(
                                    lg_psum[:, n * 512:(n + 1) * 512],
                                    w4_sb[:, p, m, :],
                                    t_bf[:, n * 512:(n + 1) * 512],
                                    start=(m == 0 and p == 0),
                                    stop=(m == MT - 1 and p == 1),
                                )

                    for m in range(MT):
                        wsb = w3pool.tile([128, KB, 2, 128], dt.float8e4, tag="w3")
                        nc.sync.dma_start(out=wsb[:], in_=w3p[m])
                        psum = ps3.tile([128, BC], dt.float32, tag="ps3")
                        for b in range(KB):
                            for n in range(NH):
                                nc.tensor.matmul(
                                    psum[:, n * 512:(n + 1) * 512],
                                    wsb[:, b],
                                    h2[:, 2 * b:2 * b + 2, n * 512:(n + 1) * 512],
                                    start=(b == 0),
                                    stop=(b == KB - 1),
                                    perf_mode=DR,
                                )
                        t_bf = h3pool.tile([128, BC], dt.bfloat16, tag="h3")
                        nc.scalar.activation(t_bf[:], psum[:, :],
                                             mybir.ActivationFunctionType.Identity,
                                             bias=b3_sb[:, m:m + 1])
                        nc.vector.tensor_scalar(t_bf[:], t_bf[:], 1.0, -1.0,
                                                mybir.AluOpType.min,
                                                mybir.AluOpType.max)
                        h3_tiles[m] = t_bf
                        # fc4 for the previous m (software-pipelined by one so
                        # the PE never waits on this m's ACT/DVE epilogue)
                        if m > 0:
                            fc4_mms(m - 1)
                    fc4_mms(MT - 1)

                # ---------------- bias + log_softmax ----------------
                lg_sb = lgsbp.tile([DOUT, BC], dt.float32)
                nc.scalar.activation(lg_sb[:], lg_psum[:],
                                     mybir.ActivationFunctionType.Identity,
                                     bias=b4_sb[:, 0:1])
                with tc.tile_pool(name="tp", bufs=2, space="PSUM") as tpp, \
                     tc.tile_pool(name="sm", bufs=2) as smp:
                    for j in range(BC // 128):
                        tp = tpp.tile([128, DOUT], dt.float32, tag="tp")
                        nc.tensor.transpose(tp[:], lg_sb[:, j * 128:(j + 1) * 128],
                                            ident[:])
                        rowmax = smp.tile([128, 1], dt.float32, tag="rmax")
                        nc.vector.tensor_reduce(rowmax[:], tp[:],
                                                mybir.AxisListType.X,
                                                mybir.AluOpType.max)
                        sh = smp.tile([128, DOUT], dt.float32, tag="sh")
                        nc.vector.tensor_scalar(sh[:], tp[:], rowmax[:], None,
                                                mybir.AluOpType.subtract)
                        ex = smp.tile([128, DOUT], dt.float32, tag="ex")
                        sums = smp.tile([128, 1], dt.float32, tag="sums")
                        nc.scalar.activation(ex[:], sh[:],
                                             mybir.ActivationFunctionType.Exp,
                                             accum_out=sums[:])
                        lns = smp.tile([128, 1], dt.float32, tag="lns")
                        nc.scalar.activation(lns[:], sums[:],
                                             mybir.ActivationFunctionType.Ln)
                        res = smp.tile([128, DOUT], dt.float32, tag="res")
                        nc.vector.tensor_scalar(res[:], sh[:], lns[:], None,
                                                mybir.AluOpType.subtract)
                        nc.sync.dma_start(out=out[j * 128:(j + 1) * 128, :],
                                          in_=res[:])

    nc.compile()
    return nc


def _pack_inputs(x, w1, b1, w2, b2, w3, b3, w4, b4):
    """Host-side packing into the device layouts. Shared tensors are packed
    once; only xt differs per core."""
    f32 = np.float32
    x = np.asarray(x, f32).reshape(B, DIN)

    # fc1 weights: sign(w1).T padded to [896, 6144], layout [k, p, m]
    s1 = np.sign(np.asarray(w1, f32))                       # [DH, DIN]
    s1t = np.zeros((DINP, DH), f32)
    s1t[:DIN] = s1.T
    w1t = np.ascontiguousarray(s1t.reshape(KT1, 128, DH)).astype(BF16)

    def pack_dr(w):
        # sign(w).T -> [mo, p, b, i, m'] DoubleRow stationary layout
        st = np.sign(np.asarray(w, f32)).T                  # [in, out]
        r = st.reshape(KB, 2, 128, MT, 128)                 # [b, i, p, mo, m']
        return np.ascontiguousarray(r.transpose(3, 2, 0, 1, 4)).astype(FP8)

    w2p = pack_dr(w2)
    w3p = pack_dr(w3)

    # fc4 weights: w4.T split hi/lo bf16, layout [p, pass, j, c]
    w4t = np.asarray(w4, f32).T                             # [DH, DOUT]
    hi = w4t.astype(BF16)
    lo = (w4t - hi.astype(f32)).astype(BF16)
    w4s = np.stack([hi, lo])                                # [2, DH, DOUT]
    w4p = np.ascontiguousarray(
        w4s.reshape(2, MT, 128, DOUT).transpose(2, 0, 1, 3))

    def pack_b(b):
        return np.ascontiguousarray(np.asarray(b, f32).reshape(MT, 128).T)

    b1p, b2p, b3p = pack_b(b1), pack_b(b2), pack_b(b3)
    b4p = np.asarray(b4, f32).reshape(DOUT, 1)

    shared = {"w1t": w1t, "w2p": w2p, "w3p": w3p, "w4p": w4p,
              "b1p": b1p, "b2p": b2p, "b3p": b3p, "b4p": b4p}

    # per-core x: 3-term bf16 split, padded, layout [p, k, s, n]
    in_maps = []
    for c in range(CORES):
        xc = x[c * BC:(c + 1) * BC]                         # [BC, DIN]
        hi = xc.astype(BF16)
        mid = (xc - hi.astype(f32)).astype(BF16)
        lo = (xc - hi.astype(f32) - mid.astype(f32)).astype(BF16)
        arr = np.zeros((3, DINP, BC), BF16)
        arr[0, :DIN] = hi.T
        arr[1, :DIN] = mid.T
        arr[2, :DIN] = lo.T
        xt = np.ascontiguousarray(
            arr.reshape(3, KT1, 128, BC).transpose(2, 1, 0, 3))
        in_maps.append({"xt": xt, **shared})
    return in_maps


_cached_nc = None


def kernel(x, w1, b1, w2, b2, w3, b3, w4, b4):
    global _cached_nc, last_exec_time_ns
    import os
    trace = bool(int(os.environ.get("KERNEL_TRACE", "0")))
    if _cached_nc is None:
        _cached_nc = _build_program()
    in_maps = _pack_inputs(x, w1, b1, w2, b2, w3, b3, w4, b4)
    res = run_bass_kernel_spmd(_cached_nc, in_maps, list(range(CORES)),
                               trace=trace)
    last_exec_time_ns = res.exec_time_ns
    return np.concatenate([res.results[c]["out"] for c in range(CORES)], axis=0)


# revision 3
# speedup vs baseline: 1.1383x; 1.1383x over previous
"""Binarized 4-layer MLP (8192x784 -> 6144 -> 6144 -> 6144 -> 10, log_softmax)
on 8 Trainium2 NeuronCores, data-parallel over the batch.

Per-core dataflow (batch slice of 1024, feature-major activations [feat, batch]):
  fc1: x @ sign(w1).T as a 2-term fp16 hi/lo split of x, with the two terms
       stacked along the contraction dim (1568 rows -> 13 k-tiles). fp16
       upconverts losslessly to the PE's e10m11 internal format and the
       weights are exactly +-1, so this reproduces fp32 accuracy.
  fc2/fc3: sign(h) @ sign(w).T in fp8e4 with DoubleRow perf mode. All products
       are +-1 and partial sums are small integers, so fp32 PSUM accumulation
       is bit-exact regardless of order.
  fc4: fused into the fc3 m-loop, single fp16 pass (w4 and h3 in fp16).
  log_softmax: PE-transpose of the [10, 1024] logits to [1024, 10] tiles,
       then max/exp/sum/ln on Vector+Scalar engines.
"""

import numpy as np
import ml_dtypes

import concourse.bass as bass
import concourse.mybir as mybir
from concourse import bacc
from concourse.tile import TileContext
from concourse.bass_utils import run_bass_kernel_spmd
from concourse.masks import make_identity

dt = mybir.dt

CORES = 8
B = 8192
BC = B // CORES          # 1024 batch rows per core
DIN = 784
KT1 = 13                 # fc1 contraction tiles: 2*784 = 1568 padded to 1664
K1P = KT1 * 128
DH = 6144
MT = DH // 128           # 48 feature tiles
KB = DH // 256           # 24 DoubleRow contraction blocks
DOUT = 10
NH = BC // 512           # 2 moving halves of 512
MQ = 4                   # fc1 m-quarters (w1 streamed per quarter)
MPQ = MT // MQ

BF16 = ml_dtypes.bfloat16
FP8 = mybir.dt.np(dt.float8e4)

last_exec_time_ns = None


def _build_program():
    nc = bacc.Bacc("TRN2", target_bir_lowering=False, debug=False,
                   num_devices=CORES)

    xt = nc.dram_tensor("xt", [128, KT1, BC], dt.float16,
                        kind="ExternalInput").ap()
    w1t = nc.dram_tensor("w1t", [KT1, 128, DH], dt.float16,
                         kind="ExternalInput").ap()
    w2p = nc.dram_tensor("w2p", [MT, 128, KB, 2, 128], dt.float8e4,
                         kind="ExternalInput").ap()
    w3p = nc.dram_tensor("w3p", [MT, 128, KB, 2, 128], dt.float8e4,
                         kind="ExternalInput").ap()
    w4p = nc.dram_tensor("w4p", [128, MT, DOUT], dt.float16,
                         kind="ExternalInput").ap()
    b1p = nc.dram_tensor("b1p", [128, MT], dt.float32, kind="ExternalInput").ap()
    b2p = nc.dram_tensor("b2p", [128, MT], dt.float32, kind="ExternalInput").ap()
    b3p = nc.dram_tensor("b3p", [128, MT], dt.float32, kind="ExternalInput").ap()
    b4p = nc.dram_tensor("b4p", [DOUT, 1], dt.float32, kind="ExternalInput").ap()
    out = nc.dram_tensor("out", [BC, DOUT], dt.float32, kind="ExternalOutput").ap()

    DR = mybir.MatmulPerfMode.DoubleRow
    AF = mybir.ActivationFunctionType

    with TileContext(nc) as tc:
        with tc.tile_pool(name="consts", bufs=1) as cpool, \
             tc.tile_pool(name="h1p", bufs=1) as h1pool:
            b1_sb = cpool.tile([128, MT], dt.float32)
            nc.sync.dma_start(out=b1_sb[:], in_=b1p[:])
            b2_sb = cpool.tile([128, MT], dt.float32)
            nc.sync.dma_start(out=b2_sb[:], in_=b2p[:])
            b3_sb = cpool.tile([128, MT], dt.float32)
            nc.sync.dma_start(out=b3_sb[:], in_=b3p[:])
            b4_sb = cpool.tile([DOUT, 1], dt.float32)
            nc.sync.dma_start(out=b4_sb[:], in_=b4p[:])
            w4_sb = cpool.tile([128, MT, DOUT], dt.float16)
            nc.sync.dma_start(out=w4_sb[:], in_=w4p[:])
            ident = cpool.tile([DOUT, DOUT], dt.float32)
            make_identity(nc, ident[:])

            h1 = h1pool.tile([128, MT, BC], dt.float8e4)

            # ---------------- fc1 ----------------
            with tc.tile_pool(name="xtp", bufs=1) as xtpool, \
                 tc.tile_pool(name="w1pool", bufs=2 * KT1) as w1pool, \
                 tc.tile_pool(name="ps1", bufs=3, space="PSUM") as ps1:
                xts = []
                for k in range(KT1):
                    t = xtpool.tile([128, BC], dt.float16, tag=f"xt{k}")
                    nc.sync.dma_start(out=t[:], in_=xt[:, k, :])
                    xts.append(t)
                for q in range(MQ):
                    w1q = []
                    for k in range(KT1):
                        t = w1pool.tile([128, MPQ * 128], dt.float16, tag="w1")
                        nc.sync.dma_start(
                            out=t[:],
                            in_=w1t[k, :, q * MPQ * 128:(q + 1) * MPQ * 128])
                        w1q.append(t)
                    for mi in range(MPQ):
                        m = q * MPQ + mi
                        psum = ps1.tile([128, BC], dt.float32, tag="ps1")
                        for k in range(KT1):
                            lhsT = w1q[k][:, mi * 128:(mi + 1) * 128]
                            for n in range(NH):
                                nc.tensor.matmul(
                                    psum[:, n * 512:(n + 1) * 512],
                                    lhsT,
                                    xts[k][:, n * 512:(n + 1) * 512],
                                    start=(k == 0),
                                    stop=(k == KT1 - 1),
                                )
                        nc.scalar.sign(h1[:, m, :], psum[:, :],
                                       bias=b1_sb[:, m:m + 1])

            # ---------------- fc2 ----------------
            with tc.tile_pool(name="h2p", bufs=1) as h2pool:
                h2 = h2pool.tile([128, MT, BC], dt.float8e4)
                with tc.tile_pool(name="w2pool", bufs=3) as w2pool, \
                     tc.tile_pool(name="ps2", bufs=3, space="PSUM") as ps2:
                    for m in range(MT):
                        wsb = w2pool.tile([128, KB, 2, 128], dt.float8e4,
                                          tag="w2")
                        nc.sync.dma_start(out=wsb[:], in_=w2p[m])
                        psum = ps2.tile([128, BC], dt.float32, tag="ps2")
                        for b in range(KB):
                            for n in range(NH):
                                nc.tensor.matmul(
                                    psum[:, n * 512:(n + 1) * 512],
                                    wsb[:, b],
                                    h1[:, 2 * b:2 * b + 2,
                                       n * 512:(n + 1) * 512],
                                    start=(b == 0),
                                    stop=(b == KB - 1),
                                    perf_mode=DR,
                                )
                        nc.scalar.sign(h2[:, m, :], psum[:, :],
                                       bias=b2_sb[:, m:m + 1])

                # ---------------- fc3 + fused fc4 ----------------
                with tc.tile_pool(name="lgp", bufs=1, space="PSUM") as lgp, \
                     tc.tile_pool(name="lgsbp", bufs=1) as lgsbp:
                    lg_psum = lgp.tile([DOUT, BC], dt.float32)
                    with tc.tile_pool(name="w3pool", bufs=3) as w3pool, \
                         tc.tile_pool(name="h3pool", bufs=3) as h3pool, \
                         tc.tile_pool(name="ps3", bufs=3, space="PSUM") as ps3:
                        h3_tiles = [None] * MT

                        def fc4_mms(m):
                            t_h3 = h3_tiles[m]
                            for n in range(NH):
                                nc.tensor.matmul(
                                    lg_psum[:, n * 512:(n + 1) * 512],
                                    w4_sb[:, m, :],
                                    t_h3[:, n * 512:(n + 1) * 512],
                                    start=(m == 0),
                                    stop=(m == MT - 1),
                                )

                        for m in range(MT):
                            wsb = w3pool.tile([128, KB, 2, 128], dt.float8e4,
                                              tag="w3")
                            nc.sync.dma_start(out=wsb[:], in_=w3p[m])
                            psum = ps3.tile([128, BC], dt.float32, tag="ps3")
                            for b in range(KB):
                                for n in range(NH):
                                    nc.tensor.matmul(
                                        psum[:, n * 512:(n + 1) * 512],
                                        wsb[:, b],
                                        h2[:, 2 * b:2 * b + 2,
                                           n * 512:(n + 1) * 512],
                                        start=(b == 0),
                                        stop=(b == KB - 1),
                                        perf_mode=DR,
                                    )
                            t_h3 = h3pool.tile([128, BC], dt.float16, tag="h3")
                            nc.scalar.activation(t_h3[:], psum[:, :],
                                                 AF.Identity,
                                                 bias=b3_sb[:, m:m + 1])
                            nc.vector.tensor_scalar(t_h3[:], t_h3[:], 1.0, -1.0,
                                                    mybir.AluOpType.min,
                                                    mybir.AluOpType.max)
                            h3_tiles[m] = t_h3
                            # fc4 for the previous m (software-pipelined by one
                            # so the PE never waits on this m's ACT/DVE)
                            if m > 0:
                                fc4_mms(m - 1)
                        fc4_mms(MT - 1)

                    # ---------------- bias + log_softmax ----------------
                    lg_sb = lgsbp.tile([DOUT, BC], dt.float32)
                    nc.scalar.activation(lg_sb[:], lg_psum[:], AF.Identity,
                                         bias=b4_sb[:, 0:1])
                    with tc.tile_pool(name="tp", bufs=2, space="PSUM") as tpp, \
                         tc.tile_pool(name="sm", bufs=2) as smp:
                        for j in range(BC // 128):
                            tp = tpp.tile([128, DOUT], dt.float32, tag="tp")
                            nc.tensor.transpose(
                                tp[:], lg_sb[:, j * 128:(j + 1) * 128], ident[:])
                            rowmax = smp.tile([128, 1], dt.float32, tag="rmax")
                            nc.vector.tensor_reduce(rowmax[:], tp[:],
                                                    mybir.AxisListType.X,
                                                    mybir.AluOpType.max)
                            sh = smp.tile([128, DOUT], dt.float32, tag="sh")
                            nc.vector.tensor_scalar(sh[:], tp[:], rowmax[:],
                                                    None,
                                                    mybir.AluOpType.subtract)
                            ex = smp.tile([128, DOUT], dt.float32, tag="ex")
                            sums = smp.tile([128, 1], dt.float32, tag="sums")
                            nc.scalar.activation(ex[:], sh[:], AF.Exp,
                                                 accum_out=sums[:])
                            lns = smp.tile([128, 1], dt.float32, tag="lns")
                            nc.scalar.activation(lns[:], sums[:], AF.Ln)
                            res = smp.tile([128, DOUT], dt.float32, tag="res")
                            nc.vector.tensor_scalar(res[:], sh[:], lns[:], None,
                                                    mybir.AluOpType.subtract)
                            nc.sync.dma_start(
                                out=out[j * 128:(j + 1) * 128, :], in_=res[:])

    nc.compile()
    return nc


def _pack_inputs(x, w1, b1, w2, b2, w3, b3, w4, b4):
    """Host-side packing into the device layouts. Shared tensors are packed
    once; only xt differs per core."""
    f32 = np.float32
    f16 = np.float16
    x = np.asarray(x, f32).reshape(B, DIN)

    # fc1 weights: sign(w1).T stacked twice (hi/lo terms share the weights),
    # padded to [1664, 6144], layout [k, p, m]
    s1 = np.sign(np.asarray(w1, f32))                       # [DH, DIN]
    s1t = np.zeros((K1P, DH), f16)
    s1t[:DIN] = s1.T
    s1t[DIN:2 * DIN] = s1.T
    w1t = np.ascontiguousarray(s1t.reshape(KT1, 128, DH))

    def pack_dr(w):
        # sign(w).T -> [mo, p, b, i, m'] DoubleRow stationary layout
        st = np.sign(np.asarray(w, f32)).T                  # [in, out]
        r = st.reshape(KB, 2, 128, MT, 128)                 # [b, i, p, mo, m']
        return np.ascontiguousarray(r.transpose(3, 2, 0, 1, 4)).astype(FP8)

    w2p = pack_dr(w2)
    w3p = pack_dr(w3)

    # fc4 weights: w4.T in fp16, layout [p, j, c]
    w4t = np.asarray(w4, f32).T.astype(f16)                 # [DH, DOUT]
    w4p = np.ascontiguousarray(w4t.reshape(MT, 128, DOUT).transpose(1, 0, 2))

    def pack_b(b):
        return np.ascontiguousarray(np.asarray(b, f32).reshape(MT, 128).T)

    b1p, b2p, b3p = pack_b(b1), pack_b(b2), pack_b(b3)
    b4p = np.asarray(b4, f32).reshape(DOUT, 1)

    shared = {"w1t": w1t, "w2p": w2p, "w3p": w3p, "w4p": w4p,
              "b1p": b1p, "b2p": b2p, "b3p": b3p, "b4p": b4p}

    # per-core x: fp16 hi/lo split stacked along contraction, layout [p, k, n]
    in_maps = []
    for c in range(CORES):
        xc = x[c * BC:(c + 1) * BC]                         # [BC, DIN]
        hi = xc.astype(f16)
        lo = (xc - hi.astype(f32)).astype(f16)
        arr = np.zeros((K1P, BC), f16)
        arr[:DIN] = hi.T
        arr[DIN:2 * DIN] = lo.T
        xt = np.ascontiguousarray(arr.reshape(KT1, 128, BC).transpose(1, 0, 2))
        in_maps.append({"xt": xt, **shared})
    return in_maps


_cached_nc = None


def kernel(x, w1, b1, w2, b2, w3, b3, w4, b4):
    global _cached_nc, last_exec_time_ns
    import os
    trace = bool(int(os.environ.get("KERNEL_TRACE", "0")))
    if _cached_nc is None:
        _cached_nc = _build_program()
    in_maps = _pack_inputs(x, w1, b1, w2, b2, w3, b3, w4, b4)
    res = run_bass_kernel_spmd(_cached_nc, in_maps, list(range(CORES)),
                               trace=trace)
    last_exec_time_ns = res.exec_time_ns
    return np.concatenate([res.results[c]["out"] for c in range(CORES)], axis=0)


# revision 4
# speedup vs baseline: 1.1440x; 1.0051x over previous
"""Binarized 4-layer MLP (8192x784 -> 6144 -> 6144 -> 6144 -> 10, log_softmax)
on 8 Trainium2 NeuronCores, data-parallel over the batch.

Per-core dataflow (batch slice of 1024, feature-major activations [feat, batch]):
  fc1: x @ sign(w1).T as a 2-term fp16 hi/lo split of x, with the two terms
       stacked along the contraction dim (1568 rows -> 13 k-tiles). fp16
       upconverts losslessly to the PE's e10m11 internal format and the
       weights are exactly +-1, so this reproduces fp32 accuracy.
  fc2/fc3: sign(h) @ sign(w).T in fp8e4 with DoubleRow perf mode. All products
       are +-1 and partial sums are small integers, so fp32 PSUM accumulation
       is bit-exact regardless of order.
  fc4: fused into the fc3 m-loop, single fp16 pass (w4 and h3 in fp16).
  log_softmax: PE-transpose of the [10, 1024] logits to [1024, 10] tiles,
       then max/exp/sum/ln on Vector+Scalar engines.
"""

import numpy as np
import ml_dtypes

import concourse.bass as bass
import concourse.mybir as mybir
from concourse import bacc
from concourse.tile import TileContext
from concourse.bass_utils import run_bass_kernel_spmd
from concourse.masks import make_identity

dt = mybir.dt

CORES = 8
B = 8192
BC = B // CORES          # 1024 batch rows per core
DIN = 784
KT1 = 13                 # fc1 contraction tiles: 2*784 = 1568 padded to 1664
K1P = KT1 * 128
DH = 6144
MT = DH // 128           # 48 feature tiles
KB = DH // 256           # 24 DoubleRow contraction blocks
DOUT = 10
NH = BC // 512           # 2 moving halves of 512
MQ = 4                   # fc1 m-quarters (w1 streamed per quarter)
MPQ = MT // MQ

BF16 = ml_dtypes.bfloat16
FP8 = mybir.dt.np(dt.float8e4)

# fc1 k-tile chunks for the first m-quarter (fine-grained so the PE can start
# as soon as the first strips land)
Q0_CHUNKS = [(0, 1), (1, 4), (5, 4), (9, 4)]

last_exec_time_ns = None


def _build_program():
    nc = bacc.Bacc("TRN2", target_bir_lowering=False, debug=False,
                   num_devices=CORES)

    xt = nc.dram_tensor("xt", [128, KT1, BC], dt.float16,
                        kind="ExternalInput").ap()
    w1t = nc.dram_tensor("w1t", [MQ, 128, KT1, MPQ * 128], dt.float16,
                         kind="ExternalInput").ap()
    w2p = nc.dram_tensor("w2p", [MT, 128, KB, 2, 128], dt.float8e4,
                         kind="ExternalInput").ap()
    w3p = nc.dram_tensor("w3p", [MT, 128, KB, 2, 128], dt.float8e4,
                         kind="ExternalInput").ap()
    w4p = nc.dram_tensor("w4p", [128, MT, DOUT], dt.float16,
                         kind="ExternalInput").ap()
    b1p = nc.dram_tensor("b1p", [128, MT], dt.float32, kind="ExternalInput").ap()
    b2p = nc.dram_tensor("b2p", [128, MT], dt.float32, kind="ExternalInput").ap()
    b3p = nc.dram_tensor("b3p", [128, MT], dt.float32, kind="ExternalInput").ap()
    b4p = nc.dram_tensor("b4p", [DOUT, 1], dt.float32, kind="ExternalInput").ap()
    out = nc.dram_tensor("out", [BC, DOUT], dt.float32, kind="ExternalOutput").ap()

    DR = mybir.MatmulPerfMode.DoubleRow
    AF = mybir.ActivationFunctionType

    with TileContext(nc) as tc:
        with tc.tile_pool(name="consts", bufs=1) as cpool, \
             tc.tile_pool(name="h1p", bufs=1) as h1pool:
            # --- startup DMAs, most-urgent first, split across two queues ---
            # fc1 first-quarter weights + x, chunked along k
            w1q0 = {}
            for (k0, nk) in Q0_CHUNKS:
                t = cpool.tile([128, nk, MPQ * 128], dt.float16, tag=f"w1q0_{k0}")
                nc.sync.dma_start(out=t[:], in_=w1t[0, :, k0:k0 + nk, :])
                for k in range(k0, k0 + nk):
                    w1q0[k] = (t, k - k0)
            xts = {}
            for (k0, nk) in Q0_CHUNKS:
                t = cpool.tile([128, nk, BC], dt.float16, tag=f"xt_{k0}")
                nc.gpsimd.dma_start(out=t[:], in_=xt[:, k0:k0 + nk, :])
                for k in range(k0, k0 + nk):
                    xts[k] = (t, k - k0)
            # first weight tiles of fc2/fc3 (avoids waiting on the SBUF zone
            # recycle at the phase boundary)
            w2f = cpool.tile([128, KB, 2, 128], dt.float8e4)
            nc.sync.dma_start(out=w2f[:], in_=w2p[0])
            w3f = cpool.tile([128, KB, 2, 128], dt.float8e4)
            nc.gpsimd.dma_start(out=w3f[:], in_=w3p[0])

            b1_sb = cpool.tile([128, MT], dt.float32)
            nc.sync.dma_start(out=b1_sb[:], in_=b1p[:])
            b2_sb = cpool.tile([128, MT], dt.float32)
            nc.gpsimd.dma_start(out=b2_sb[:], in_=b2p[:])
            b3_sb = cpool.tile([128, MT], dt.float32)
            nc.sync.dma_start(out=b3_sb[:], in_=b3p[:])
            b4_sb = cpool.tile([DOUT, 1], dt.float32)
            nc.gpsimd.dma_start(out=b4_sb[:], in_=b4p[:])
            w4_sb = cpool.tile([128, MT, DOUT], dt.float16)
            nc.sync.dma_start(out=w4_sb[:], in_=w4p[:])
            ident = cpool.tile([DOUT, DOUT], dt.float32)
            make_identity(nc, ident[:])
            # pre-warm the Exp/Ln activation tables so the log_softmax tail
            # doesn't pay the ~1.3us ACT_TABLE_LOADs serially
            warm = cpool.tile([1, 1], dt.float32)
            nc.scalar.activation(warm[:], ident[0:1, 0:1], AF.Exp)
            nc.scalar.activation(warm[:], warm[:], AF.Ln)

            h1 = h1pool.tile([128, MT, BC], dt.float8e4)

            # ---------------- fc1 ----------------
            with tc.tile_pool(name="w1pool", bufs=2) as w1pool, \
                 tc.tile_pool(name="ps1", bufs=3, space="PSUM") as ps1:
                for q in range(MQ):
                    if q == 0:
                        def lhs1(k, mi):
                            t, kk = w1q0[k]
                            return t[:, kk, mi * 128:(mi + 1) * 128]
                    else:
                        w1q = w1pool.tile([128, KT1, MPQ * 128], dt.float16,
                                          tag="w1")
                        nc.sync.dma_start(out=w1q[:], in_=w1t[q])

                        def lhs1(k, mi, w1q=w1q):
                            return w1q[:, k, mi * 128:(mi + 1) * 128]
                    for mi in range(MPQ):
                        m = q * MPQ + mi
                        psum = ps1.tile([128, BC], dt.float32, tag="ps1")
                        for k in range(KT1):
                            xtile, xk = xts[k]
                            for n in range(NH):
                                nc.tensor.matmul(
                                    psum[:, n * 512:(n + 1) * 512],
                                    lhs1(k, mi),
                                    xtile[:, xk, n * 512:(n + 1) * 512],
                                    start=(k == 0),
                                    stop=(k == KT1 - 1),
                                )
                        nc.scalar.sign(h1[:, m, :], psum[:, :],
                                       bias=b1_sb[:, m:m + 1])

            # ---------------- fc2 ----------------
            with tc.tile_pool(name="h2p", bufs=1) as h2pool:
                h2 = h2pool.tile([128, MT, BC], dt.float8e4)
                with tc.tile_pool(name="w2pool", bufs=3) as w2pool, \
                     tc.tile_pool(name="ps2", bufs=3, space="PSUM") as ps2:
                    for m in range(MT):
                        if m == 0:
                            wsb = w2f
                        else:
                            wsb = w2pool.tile([128, KB, 2, 128], dt.float8e4,
                                              tag="w2")
                            nc.sync.dma_start(out=wsb[:], in_=w2p[m])
                        psum = ps2.tile([128, BC], dt.float32, tag="ps2")
                        for b in range(KB):
                            for n in range(NH):
                                nc.tensor.matmul(
                                    psum[:, n * 512:(n + 1) * 512],
                                    wsb[:, b],
                                    h1[:, 2 * b:2 * b + 2,
                                       n * 512:(n + 1) * 512],
                                    start=(b == 0),
                                    stop=(b == KB - 1),
                                    perf_mode=DR,
                                )
                        nc.scalar.sign(h2[:, m, :], psum[:, :],
                                       bias=b2_sb[:, m:m + 1])

                # ---------------- fc3 + fused fc4 ----------------
                with tc.tile_pool(name="lgp", bufs=1, space="PSUM") as lgp, \
                     tc.tile_pool(name="lgsbp", bufs=1) as lgsbp:
                    lg_psum = lgp.tile([DOUT, BC], dt.float32)
                    with tc.tile_pool(name="w3pool", bufs=3) as w3pool, \
                         tc.tile_pool(name="h3pool", bufs=3) as h3pool, \
                         tc.tile_pool(name="ps3", bufs=3, space="PSUM") as ps3:
                        h3_tiles = [None] * MT

                        def fc4_mms(m):
                            t_h3 = h3_tiles[m]
                            for n in range(NH):
                                nc.tensor.matmul(
                                    lg_psum[:, n * 512:(n + 1) * 512],
                                    w4_sb[:, m, :],
                                    t_h3[:, n * 512:(n + 1) * 512],
                                    start=(m == 0),
                                    stop=(m == MT - 1),
                                )

                        for m in range(MT):
                            if m == 0:
                                wsb = w3f
                            else:
                                wsb = w3pool.tile([128, KB, 2, 128],
                                                  dt.float8e4, tag="w3")
                                nc.sync.dma_start(out=wsb[:], in_=w3p[m])
                            psum = ps3.tile([128, BC], dt.float32, tag="ps3")
                            for b in range(KB):
                                for n in range(NH):
                                    nc.tensor.matmul(
                                        psum[:, n * 512:(n + 1) * 512],
                                        wsb[:, b],
                                        h2[:, 2 * b:2 * b + 2,
                                           n * 512:(n + 1) * 512],
                                        start=(b == 0),
                                        stop=(b == KB - 1),
                                        perf_mode=DR,
                                    )
                            t_h3 = h3pool.tile([128, BC], dt.float16, tag="h3")
                            nc.scalar.activation(t_h3[:], psum[:, :],
                                                 AF.Identity,
                                                 bias=b3_sb[:, m:m + 1])
                            nc.vector.tensor_scalar(t_h3[:], t_h3[:], 1.0, -1.0,
                                                    mybir.AluOpType.min,
                                                    mybir.AluOpType.max)
                            h3_tiles[m] = t_h3
                            # fc4 for the previous m (software-pipelined by one
                            # so the PE never waits on this m's ACT/DVE)
                            if m > 0:
                                fc4_mms(m - 1)
                        fc4_mms(MT - 1)

                    # ---------------- bias + log_softmax ----------------
                    lg_sb = lgsbp.tile([DOUT, BC], dt.float32)
                    nc.scalar.activation(lg_sb[:], lg_psum[:], AF.Identity,
                                         bias=b4_sb[:, 0:1])
                    with tc.tile_pool(name="tp", bufs=4, space="PSUM") as tpp, \
                         tc.tile_pool(name="sm", bufs=4) as smp:
                        for j in range(BC // 128):
                            tp = tpp.tile([128, DOUT], dt.float32, tag="tp")
                            nc.tensor.transpose(
                                tp[:], lg_sb[:, j * 128:(j + 1) * 128], ident[:])
                            rowmax = smp.tile([128, 1], dt.float32, tag="rmax")
                            nc.vector.tensor_reduce(rowmax[:], tp[:],
                                                    mybir.AxisListType.X,
                                                    mybir.AluOpType.max)
                            sh = smp.tile([128, DOUT], dt.float32, tag="sh")
                            nc.vector.tensor_scalar(sh[:], tp[:], rowmax[:],
                                                    None,
                                                    mybir.AluOpType.subtract)
                            ex = smp.tile([128, DOUT], dt.float32, tag="ex")
                            sums = smp.tile([128, 1], dt.float32, tag="sums")
                            nc.scalar.activation(ex[:], sh[:], AF.Exp,
                                                 accum_out=sums[:])
                            lns = smp.tile([128, 1], dt.float32, tag="lns")
                            nc.scalar.activation(lns[:], sums[:], AF.Ln)
                            res = smp.tile([128, DOUT], dt.float32, tag="res")
                            nc.vector.tensor_scalar(res[:], sh[:], lns[:], None,
                                                    mybir.AluOpType.subtract)
                            nc.sync.dma_start(
                                out=out[j * 128:(j + 1) * 128, :], in_=res[:])

    nc.compile()
    return nc


def _pack_inputs(x, w1, b1, w2, b2, w3, b3, w4, b4):
    """Host-side packing into the device layouts. Shared tensors are packed
    once; only xt differs per core."""
    f32 = np.float32
    f16 = np.float16
    x = np.asarray(x, f32).reshape(B, DIN)

    # fc1 weights: sign(w1).T stacked twice (hi/lo terms share the weights),
    # padded to [1664, 6144], layout [q, p, k, m]
    s1 = np.sign(np.asarray(w1, f32))                       # [DH, DIN]
    s1t = np.zeros((K1P, DH), f16)
    s1t[:DIN] = s1.T
    s1t[DIN:2 * DIN] = s1.T
    w1t = np.ascontiguousarray(
        s1t.reshape(KT1, 128, MQ, MPQ * 128).transpose(2, 1, 0, 3))

    def pack_dr(w):
        # sign(w).T -> [mo, p, b, i, m'] DoubleRow stationary layout
        st = np.sign(np.asarray(w, f32)).T                  # [in, out]
        r = st.reshape(KB, 2, 128, MT, 128)                 # [b, i, p, mo, m']
        return np.ascontiguousarray(r.transpose(3, 2, 0, 1, 4)).astype(FP8)

    w2p = pack_dr(w2)
    w3p = pack_dr(w3)

    # fc4 weights: w4.T in fp16, layout [p, j, c]
    w4t = np.asarray(w4, f32).T.astype(f16)                 # [DH, DOUT]
    w4p = np.ascontiguousarray(w4t.reshape(MT, 128, DOUT).transpose(1, 0, 2))

    def pack_b(b):
        return np.ascontiguousarray(np.asarray(b, f32).reshape(MT, 128).T)

    b1p, b2p, b3p = pack_b(b1), pack_b(b2), pack_b(b3)
    b4p = np.asarray(b4, f32).reshape(DOUT, 1)

    shared = {"w1t": w1t, "w2p": w2p, "w3p": w3p, "w4p": w4p,
              "b1p": b1p, "b2p": b2p, "b3p": b3p, "b4p": b4p}

    # per-core x: fp16 hi/lo split stacked along contraction, layout [p, k, n]
    in_maps = []
    for c in range(CORES):
        xc = x[c * BC:(c + 1) * BC]                         # [BC, DIN]
        hi = xc.astype(f16)
        lo = (xc - hi.astype(f32)).astype(f16)
        arr = np.zeros((K1P, BC), f16)
        arr[:DIN] = hi.T
        arr[DIN:2 * DIN] = lo.T
        xt = np.ascontiguousarray(arr.reshape(KT1, 128, BC).transpose(1, 0, 2))
        in_maps.append({"xt": xt, **shared})
    return in_maps


_cached_nc = None


def kernel(x, w1, b1, w2, b2, w3, b3, w4, b4):
    global _cached_nc, last_exec_time_ns
    import os
    trace = bool(int(os.environ.get("KERNEL_TRACE", "0")))
    if _cached_nc is None:
        _cached_nc = _build_program()
    in_maps = _pack_inputs(x, w1, b1, w2, b2, w3, b3, w4, b4)
    res = run_bass_kernel_spmd(_cached_nc, in_maps, list(range(CORES)),
                               trace=trace)
    last_exec_time_ns = res.exec_time_ns
    return np.concatenate([res.results[c]["out"] for c in range(CORES)], axis=0)


# revision 6
# speedup vs baseline: 1.1631x; 1.0166x over previous
"""Binarized 4-layer MLP (8192x784 -> 6144 -> 6144 -> 6144 -> 10, log_softmax)
on 8 Trainium2 NeuronCores, data-parallel over the batch.

Per-core dataflow (batch slice of 1024, feature-major activations [feat, batch]):
  fc1: x @ sign(w1).T as a 2-term fp16 hi/lo split of x, with the two terms
       stacked along the contraction dim (1568 rows -> 13 k-tiles). fp16
       upconverts losslessly to the PE's e10m11 internal format and the
       weights are exactly +-1, so this reproduces fp32 accuracy.
  fc2/fc3: sign(h) @ sign(w).T in fp8e4 with DoubleRow perf mode. All products
       are +-1 and partial sums are small integers, so fp32 PSUM accumulation
       is bit-exact regardless of order.
  fc4: fused into the fc3 m-loop, single fp16 pass (w4 and h3 in fp16).
  log_softmax: PE-transpose of the [10, 1024] logits to [1024, 10] tiles,
       then max/exp/sum/ln on Vector+Scalar engines.
"""

import numpy as np
import ml_dtypes

import concourse.bass as bass
import concourse.mybir as mybir
from concourse import bacc
from concourse.tile import TileContext
from concourse.bass_utils import run_bass_kernel_spmd
from concourse.masks import make_identity

dt = mybir.dt

CORES = 8
B = 8192
BC = B // CORES          # 1024 batch rows per core
DIN = 784
KT1 = 13                 # fc1 contraction tiles: 2*784 = 1568 padded to 1664
K1P = KT1 * 128
DH = 6144
MT = DH // 128           # 48 feature tiles
KB = DH // 256           # 24 DoubleRow contraction blocks
DOUT = 10
NH = BC // 512           # 2 moving halves of 512
MQ = 12                  # fc1 m-groups (w1 streamed per 4 m-tiles)
MPQ = MT // MQ

BF16 = ml_dtypes.bfloat16
FP8 = mybir.dt.np(dt.float8e4)

last_exec_time_ns = None


def _build_program():
    nc = bacc.Bacc("TRN2", target_bir_lowering=False, debug=False,
                   num_devices=CORES)

    xt = nc.dram_tensor("xt", [128, KT1, BC], dt.float16,
                        kind="ExternalInput").ap()
    w1t = nc.dram_tensor("w1t", [MQ, 128, KT1, MPQ * 128], dt.float16,
                         kind="ExternalInput").ap()
    w2p = nc.dram_tensor("w2p", [MT, 128, KB, 2, 128], dt.float8e4,
                         kind="ExternalInput").ap()
    w3p = nc.dram_tensor("w3p", [MT, 128, KB, 2, 128], dt.float8e4,
                         kind="ExternalInput").ap()
    w4p = nc.dram_tensor("w4p", [128, MT, DOUT], dt.float16,
                         kind="ExternalInput").ap()
    b1p = nc.dram_tensor("b1p", [128, MT], dt.float32, kind="ExternalInput").ap()
    b2p = nc.dram_tensor("b2p", [128, MT], dt.float32, kind="ExternalInput").ap()
    b3p = nc.dram_tensor("b3p", [128, MT], dt.float32, kind="ExternalInput").ap()
    b4p = nc.dram_tensor("b4p", [DOUT, 1], dt.float32, kind="ExternalInput").ap()
    out = nc.dram_tensor("out", [BC, DOUT], dt.float32, kind="ExternalOutput").ap()

    DR = mybir.MatmulPerfMode.DoubleRow
    AF = mybir.ActivationFunctionType

    with TileContext(nc) as tc:
        with tc.tile_pool(name="consts", bufs=1) as cpool, \
             tc.tile_pool(name="h1p", bufs=1) as h1pool:
            # --- startup DMAs in fc1 consumption order (k-interleaved),
            # alternating dispatch engines to halve queue serialization ---
            w1q0 = {}
            xts = {}
            for k in range(KT1):
                tx = cpool.tile([128, BC], dt.float16, tag=f"xt_{k}")
                nc.sync.dma_start(out=tx[:], in_=xt[:, k, :])
                xts[k] = tx
                tw = cpool.tile([128, MPQ * 128], dt.float16, tag=f"w1q0_{k}")
                nc.gpsimd.dma_start(out=tw[:], in_=w1t[0, :, k, :])
                w1q0[k] = tw
            # first weight tiles of fc2/fc3 (avoids waiting on the SBUF zone
            # recycle at the phase boundary)
            w2f = cpool.tile([128, KB, 2, 128], dt.float8e4)
            nc.sync.dma_start(out=w2f[:], in_=w2p[0])
            w3f = cpool.tile([128, KB, 2, 128], dt.float8e4)
            nc.gpsimd.dma_start(out=w3f[:], in_=w3p[0])

            b1_sb = cpool.tile([128, MT], dt.float32)
            nc.sync.dma_start(out=b1_sb[:], in_=b1p[:])
            b2_sb = cpool.tile([128, MT], dt.float32)
            nc.gpsimd.dma_start(out=b2_sb[:], in_=b2p[:])
            b3_sb = cpool.tile([128, MT], dt.float32)
            nc.sync.dma_start(out=b3_sb[:], in_=b3p[:])
            b4_sb = cpool.tile([DOUT, 1], dt.float32)
            nc.gpsimd.dma_start(out=b4_sb[:], in_=b4p[:])
            w4_sb = cpool.tile([128, MT, DOUT], dt.float16)
            nc.sync.dma_start(out=w4_sb[:], in_=w4p[:])
            ident = cpool.tile([DOUT, DOUT], dt.float32)
            make_identity(nc, ident[:])
            # pre-warm the Exp/Ln activation tables so the log_softmax tail
            # doesn't pay the ~1.3us ACT_TABLE_LOADs serially
            warm = cpool.tile([1, 1], dt.float32)
            nc.scalar.activation(warm[:], ident[0:1, 0:1], AF.Exp)
            nc.scalar.activation(warm[:], warm[:], AF.Ln)

            h1 = h1pool.tile([128, MT, BC], dt.float8e4)

            # ---------------- fc1 ----------------
            with tc.tile_pool(name="w1pool", bufs=3) as w1pool, \
                 tc.tile_pool(name="ps1", bufs=3, space="PSUM") as ps1:
                for q in range(MQ):
                    if q == 0:
                        def lhs1(k, mi):
                            return w1q0[k][:, mi * 128:(mi + 1) * 128]
                    else:
                        w1q = w1pool.tile([128, KT1, MPQ * 128], dt.float16,
                                          tag="w1")
                        nc.sync.dma_start(out=w1q[:], in_=w1t[q])

                        def lhs1(k, mi, w1q=w1q):
                            return w1q[:, k, mi * 128:(mi + 1) * 128]
                    for mi in range(MPQ):
                        m = q * MPQ + mi
                        psum = ps1.tile([128, BC], dt.float32, tag="ps1")
                        for k in range(KT1):
                            for n in range(NH):
                                nc.tensor.matmul(
                                    psum[:, n * 512:(n + 1) * 512],
                                    lhs1(k, mi),
                                    xts[k][:, n * 512:(n + 1) * 512],
                                    start=(k == 0),
                                    stop=(k == KT1 - 1),
                                )
                        nc.scalar.sign(h1[:, m, :], psum[:, :],
                                       bias=b1_sb[:, m:m + 1])

            # ---------------- fc2 ----------------
            with tc.tile_pool(name="h2p", bufs=1) as h2pool:
                h2 = h2pool.tile([128, MT, BC], dt.float8e4)
                with tc.tile_pool(name="w2pool", bufs=3) as w2pool, \
                     tc.tile_pool(name="ps2", bufs=3, space="PSUM") as ps2:
                    for m in range(MT):
                        if m == 0:
                            wsb = w2f
                        else:
                            wsb = w2pool.tile([128, KB, 2, 128], dt.float8e4,
                                              tag="w2")
                            nc.sync.dma_start(out=wsb[:], in_=w2p[m])
                        psum = ps2.tile([128, BC], dt.float32, tag="ps2")
                        for n in range(NH):
                            for b in range(KB):
                                nc.tensor.matmul(
                                    psum[:, n * 512:(n + 1) * 512],
                                    wsb[:, b],
                                    h1[:, 2 * b:2 * b + 2,
                                       n * 512:(n + 1) * 512],
                                    start=(b == 0),
                                    stop=(b == KB - 1),
                                    perf_mode=DR,
                                )
                        nc.scalar.sign(h2[:, m, :], psum[:, :],
                                       bias=b2_sb[:, m:m + 1])

                # ---------------- fc3 + fused fc4 ----------------
                with tc.tile_pool(name="lgp", bufs=1, space="PSUM") as lgp, \
                     tc.tile_pool(name="lgsbp", bufs=1) as lgsbp:
                    lg_psum = lgp.tile([DOUT, BC], dt.float32)
                    with tc.tile_pool(name="w3pool", bufs=3) as w3pool, \
                         tc.tile_pool(name="h3pool", bufs=18) as h3pool, \
                         tc.tile_pool(name="ps3", bufs=3, space="PSUM") as ps3:
                        h3_tiles = [None] * MT

                        def fc4_mms(m):
                            t_h3 = h3_tiles[m]
                            for n in range(NH):
                                nc.tensor.matmul(
                                    lg_psum[:, n * 512:(n + 1) * 512],
                                    w4_sb[:, m, :],
                                    t_h3[:, n * 512:(n + 1) * 512],
                                    start=(m == 0),
                                    stop=(m == MT - 1),
                                )

                        for m in range(MT):
                            if m == 0:
                                wsb = w3f
                            else:
                                wsb = w3pool.tile([128, KB, 2, 128],
                                                  dt.float8e4, tag="w3")
                                nc.sync.dma_start(out=wsb[:], in_=w3p[m])
                            psum = ps3.tile([128, BC], dt.float32, tag="ps3")
                            for n in range(NH):
                                for b in range(KB):
                                    nc.tensor.matmul(
                                        psum[:, n * 512:(n + 1) * 512],
                                        wsb[:, b],
                                        h2[:, 2 * b:2 * b + 2,
                                           n * 512:(n + 1) * 512],
                                        start=(b == 0),
                                        stop=(b == KB - 1),
                                        perf_mode=DR,
                                    )
                            t_h3 = h3pool.tile([128, BC], dt.float16, tag="h3")
                            nc.scalar.activation(t_h3[:], psum[:, :],
                                                 AF.Identity,
                                                 bias=b3_sb[:, m:m + 1])
                            nc.vector.tensor_scalar(t_h3[:], t_h3[:], 1.0, -1.0,
                                                    mybir.AluOpType.min,
                                                    mybir.AluOpType.max)
                            h3_tiles[m] = t_h3
                            # fc4 batched every 8 m-tiles (fewer stationary /
                            # perf-mode switches on the PE), pipelined one m
                            # behind so the PE never waits on this m's ACT/DVE
                            if m % 8 == 7 and m >= 15:
                                for mm in range(m - 15, m - 7):
                                    fc4_mms(mm)
                        for mm in range(MT - 8, MT):
                            fc4_mms(mm)

                    # ---------------- bias + log_softmax ----------------
                    lg_sb = lgsbp.tile([DOUT, BC], dt.float32)
                    nc.scalar.activation(lg_sb[:], lg_psum[:], AF.Identity,
                                         bias=b4_sb[:, 0:1])
                    with tc.tile_pool(name="tp", bufs=4, space="PSUM") as tpp, \
                         tc.tile_pool(name="sm", bufs=4) as smp:
                        for j in range(BC // 128):
                            tp = tpp.tile([128, DOUT], dt.float32, tag="tp")
                            nc.tensor.transpose(
                                tp[:], lg_sb[:, j * 128:(j + 1) * 128], ident[:])
                            rowmax = smp.tile([128, 1], dt.float32, tag="rmax")
                            nc.vector.tensor_reduce(rowmax[:], tp[:],
                                                    mybir.AxisListType.X,
                                                    mybir.AluOpType.max)
                            sh = smp.tile([128, DOUT], dt.float32, tag="sh")
                            nc.vector.tensor_scalar(sh[:], tp[:], rowmax[:],
                                                    None,
                                                    mybir.AluOpType.subtract)
                            ex = smp.tile([128, DOUT], dt.float32, tag="ex")
                            sums = smp.tile([128, 1], dt.float32, tag="sums")
                            nc.scalar.activation(ex[:], sh[:], AF.Exp,
                                                 accum_out=sums[:])
                            lns = smp.tile([128, 1], dt.float32, tag="lns")
                            nc.scalar.activation(lns[:], sums[:], AF.Ln)
                            res = smp.tile([128, DOUT], dt.float32, tag="res")
                            nc.vector.tensor_scalar(res[:], sh[:], lns[:], None,
                                                    mybir.AluOpType.subtract)
                            nc.sync.dma_start(
                                out=out[j * 128:(j + 1) * 128, :], in_=res[:])

    nc.compile()
    return nc


def _pack_inputs(x, w1, b1, w2, b2, w3, b3, w4, b4):
    """Host-side packing into the device layouts. Shared tensors are packed
    once; only xt differs per core."""
    f32 = np.float32
    f16 = np.float16
    x = np.asarray(x, f32).reshape(B, DIN)

    # fc1 weights: sign(w1).T stacked twice (hi/lo terms share the weights),
    # padded to [1664, 6144], layout [q, p, k, m]
    s1 = np.sign(np.asarray(w1, f32))                       # [DH, DIN]
    s1t = np.zeros((K1P, DH), f16)
    s1t[:DIN] = s1.T
    s1t[DIN:2 * DIN] = s1.T
    w1t = np.ascontiguousarray(
        s1t.reshape(KT1, 128, MQ, MPQ * 128).transpose(2, 1, 0, 3))

    def pack_dr(w):
        # sign(w).T -> [mo, p, b, i, m'] DoubleRow stationary layout
        st = np.sign(np.asarray(w, f32)).T                  # [in, out]
        r = st.reshape(KB, 2, 128, MT, 128)                 # [b, i, p, mo, m']
        return np.ascontiguousarray(r.transpose(3, 2, 0, 1, 4)).astype(FP8)

    w2p = pack_dr(w2)
    w3p = pack_dr(w3)

    # fc4 weights: w4.T in fp16, layout [p, j, c]
    w4t = np.asarray(w4, f32).T.astype(f16)                 # [DH, DOUT]
    w4p = np.ascontiguousarray(w4t.reshape(MT, 128, DOUT).transpose(1, 0, 2))

    def pack_b(b):
        return np.ascontiguousarray(np.asarray(b, f32).reshape(MT, 128).T)

    b1p, b2p, b3p = pack_b(b1), pack_b(b2), pack_b(b3)
    b4p = np.asarray(b4, f32).reshape(DOUT, 1)

    shared = {"w1t": w1t, "w2p": w2p, "w3p": w3p, "w4p": w4p,
              "b1p": b1p, "b2p": b2p, "b3p": b3p, "b4p": b4p}

    # per-core x: fp16 hi/lo split stacked along contraction, layout [p, k, n]
    in_maps = []
    for c in range(CORES):
        xc = x[c * BC:(c + 1) * BC]                         # [BC, DIN]
        hi = xc.astype(f16)
        lo = (xc - hi.astype(f32)).astype(f16)
        arr = np.zeros((K1P, BC), f16)
        arr[:DIN] = hi.T
        arr[DIN:2 * DIN] = lo.T
        xt = np.ascontiguousarray(arr.reshape(KT1, 128, BC).transpose(1, 0, 2))
        in_maps.append({"xt": xt, **shared})
    return in_maps


_cached_nc = None


def kernel(x, w1, b1, w2, b2, w3, b3, w4, b4):
    global _cached_nc, last_exec_time_ns
    import os
    trace = bool(int(os.environ.get("KERNEL_TRACE", "0")))
    if _cached_nc is None:
        _cached_nc = _build_program()
    in_maps = _pack_inputs(x, w1, b1, w2, b2, w3, b3, w4, b4)
    res = run_bass_kernel_spmd(_cached_nc, in_maps, list(range(CORES)),
                               trace=trace)
    last_exec_time_ns = res.exec_time_ns
    return np.concatenate([res.results[c]["out"] for c in range(CORES)], axis=0)


# revision 9
# speedup vs baseline: 1.1731x; 1.0086x over previous
"""Binarized 4-layer MLP (8192x784 -> 6144 -> 6144 -> 6144 -> 10, log_softmax)
on 8 Trainium2 NeuronCores, data-parallel over the batch.

Per-core dataflow (batch slice of 1024, feature-major activations [feat, batch]):
  fc1: x @ sign(w1).T as a 2-term fp16 hi/lo split of x, with the two terms
       stacked along the contraction dim (1568 rows -> 13 k-tiles). fp16
       upconverts losslessly to the PE's e10m11 internal format and the
       weights are exactly +-1, so this reproduces fp32 accuracy.
  fc2/fc3: sign(h) @ sign(w).T in fp8e4 with DoubleRow perf mode. All products
       are +-1 and partial sums are small integers, so fp32 PSUM accumulation
       is bit-exact regardless of order.
  fc4: fused into the fc3 m-loop, single fp16 pass (w4 and h3 in fp16).
  log_softmax: PE-transpose of the [10, 1024] logits to [1024, 10] tiles,
       then max/exp/sum/ln on Vector+Scalar engines.
"""

import numpy as np
import ml_dtypes

import concourse.bass as bass
import concourse.mybir as mybir
from concourse import bacc
from concourse.tile import TileContext
from concourse.bass_utils import run_bass_kernel_spmd
from concourse.masks import make_identity

dt = mybir.dt

CORES = 8
B = 8192
BC = B // CORES          # 1024 batch rows per core
DIN = 784
KT1 = 13                 # fc1 contraction tiles: 2*784 = 1568 padded to 1664
K1P = KT1 * 128
DH = 6144
MT = DH // 128           # 48 feature tiles
KB = DH // 256           # 24 DoubleRow contraction blocks
DOUT = 10
NH = BC // 512           # 2 moving halves of 512
MQ = 12                  # fc1 m-groups (w1 streamed per 4 m-tiles)
MPQ = MT // MQ

BF16 = ml_dtypes.bfloat16
FP8 = mybir.dt.np(dt.float8e4)

last_exec_time_ns = None


def _build_program():
    nc = bacc.Bacc("TRN2", target_bir_lowering=False, debug=False,
                   num_devices=CORES)

    xt = nc.dram_tensor("xt", [128, KT1, BC], dt.float16,
                        kind="ExternalInput").ap()
    w1t = nc.dram_tensor("w1t", [MQ, 128, KT1, MPQ * 128], dt.float16,
                         kind="ExternalInput").ap()
    w2p = nc.dram_tensor("w2p", [MT, 128, KB, 2, 128], dt.float8e4,
                         kind="ExternalInput").ap()
    w3p = nc.dram_tensor("w3p", [MT, 128, KB, 2, 128], dt.float8e4,
                         kind="ExternalInput").ap()
    w4p = nc.dram_tensor("w4p", [128, MT, DOUT], dt.float16,
                         kind="ExternalInput").ap()
    b1p = nc.dram_tensor("b1p", [128, MT], dt.float32, kind="ExternalInput").ap()
    b2p = nc.dram_tensor("b2p", [128, MT], dt.float32, kind="ExternalInput").ap()
    b3p = nc.dram_tensor("b3p", [128, MT], dt.float32, kind="ExternalInput").ap()
    b4p = nc.dram_tensor("b4p", [DOUT, 1], dt.float32, kind="ExternalInput").ap()
    out = nc.dram_tensor("out", [BC, DOUT], dt.float32, kind="ExternalOutput").ap()

    DR = mybir.MatmulPerfMode.DoubleRow
    AF = mybir.ActivationFunctionType

    with TileContext(nc) as tc:
        with tc.tile_pool(name="consts", bufs=1) as cpool, \
             tc.tile_pool(name="h1p", bufs=1) as h1pool:
            # --- startup DMAs in fc1 consumption order (k-interleaved),
            # alternating dispatch engines to halve queue serialization ---
            w1q0 = {}
            xt_half = {}
            for k in range(KT1):
                if k == 0:
                    for n in range(NH):
                        tx = cpool.tile([128, 512], dt.float16, tag=f"xt0_{n}")
                        nc.sync.dma_start(out=tx[:],
                                          in_=xt[:, 0, n * 512:(n + 1) * 512])
                        xt_half[(0, n)] = tx[:, :]
                else:
                    tx = cpool.tile([128, BC], dt.float16, tag=f"xt_{k}")
                    nc.sync.dma_start(out=tx[:], in_=xt[:, k, :])
                    for n in range(NH):
                        xt_half[(k, n)] = tx[:, n * 512:(n + 1) * 512]
                tw = cpool.tile([128, MPQ * 128], dt.float16, tag=f"w1q0_{k}")
                nc.gpsimd.dma_start(out=tw[:], in_=w1t[0, :, k, :])
                w1q0[k] = tw
            # first weight tiles of fc2/fc3 (avoids waiting on the SBUF zone
            # recycle at the phase boundary)
            w2f = cpool.tile([128, KB, 2, 128], dt.float8e4)
            nc.sync.dma_start(out=w2f[:], in_=w2p[0])
            w3f = cpool.tile([128, KB, 2, 128], dt.float8e4)
            nc.gpsimd.dma_start(out=w3f[:], in_=w3p[0])

            b1_sb = cpool.tile([128, MT], dt.float32)
            nc.sync.dma_start(out=b1_sb[:], in_=b1p[:])
            b2_sb = cpool.tile([128, MT], dt.float32)
            nc.gpsimd.dma_start(out=b2_sb[:], in_=b2p[:])
            b3_sb = cpool.tile([128, MT], dt.float32)
            nc.sync.dma_start(out=b3_sb[:], in_=b3p[:])
            b4_sb = cpool.tile([DOUT, 1], dt.float32)
            nc.gpsimd.dma_start(out=b4_sb[:], in_=b4p[:])
            w4_sb = cpool.tile([128, MT, DOUT], dt.float16)
            nc.sync.dma_start(out=w4_sb[:], in_=w4p[:])
            ident = cpool.tile([DOUT, DOUT], dt.float32)
            make_identity(nc, ident[:])
            # pre-warm the Exp/Ln activation tables so the log_softmax tail
            # doesn't pay the ~1.3us ACT_TABLE_LOADs serially
            warm = cpool.tile([1, 1], dt.float32)
            nc.scalar.activation(warm[:], ident[0:1, 0:1], AF.Exp)
            nc.scalar.activation(warm[:], warm[:], AF.Ln)

            h1 = h1pool.tile([128, MT, BC], dt.float8e4)

            # ---------------- fc1 ----------------
            with tc.tile_pool(name="w1pool", bufs=3) as w1pool, \
                 tc.tile_pool(name="ps1", bufs=3, space="PSUM") as ps1:
                for q in range(MQ):
                    if q == 0:
                        def lhs1(k, mi):
                            return w1q0[k][:, mi * 128:(mi + 1) * 128]
                    else:
                        w1q = w1pool.tile([128, KT1, MPQ * 128], dt.float16,
                                          tag="w1")
                        nc.sync.dma_start(out=w1q[:], in_=w1t[q])

                        def lhs1(k, mi, w1q=w1q):
                            return w1q[:, k, mi * 128:(mi + 1) * 128]
                    for mi in range(MPQ):
                        m = q * MPQ + mi
                        psum = ps1.tile([128, BC], dt.float32, tag="ps1")
                        for k in range(KT1):
                            for n in range(NH):
                                nc.tensor.matmul(
                                    psum[:, n * 512:(n + 1) * 512],
                                    lhs1(k, mi),
                                    xt_half[(k, n)],
                                    start=(k == 0),
                                    stop=(k == KT1 - 1),
                                )
                        nc.scalar.sign(h1[:, m, :], psum[:, :],
                                       bias=b1_sb[:, m:m + 1])

            # ---------------- fc2 ----------------
            with tc.tile_pool(name="h2p", bufs=1) as h2pool:
                h2 = h2pool.tile([128, MT, BC], dt.float8e4)
                with tc.tile_pool(name="w2pool", bufs=3) as w2pool, \
                     tc.tile_pool(name="ps2", bufs=3, space="PSUM") as ps2:
                    for m in range(MT):
                        if m == 0:
                            wsb = w2f
                        else:
                            wsb = w2pool.tile([128, KB, 2, 128], dt.float8e4,
                                              tag="w2")
                            nc.sync.dma_start(out=wsb[:], in_=w2p[m])
                        psum = ps2.tile([128, BC], dt.float32, tag="ps2")
                        for n in range(NH):
                            for b in range(KB):
                                nc.tensor.matmul(
                                    psum[:, n * 512:(n + 1) * 512],
                                    wsb[:, b],
                                    h1[:, 2 * b:2 * b + 2,
                                       n * 512:(n + 1) * 512],
                                    start=(b == 0),
                                    stop=(b == KB - 1),
                                    perf_mode=DR,
                                )
                        nc.scalar.sign(h2[:, m, :], psum[:, :],
                                       bias=b2_sb[:, m:m + 1])

                # ---------------- fc3 + fused fc4 ----------------
                with tc.tile_pool(name="lgp", bufs=1, space="PSUM") as lgp, \
                     tc.tile_pool(name="lgsbp", bufs=1) as lgsbp:
                    lg_psum = lgp.tile([DOUT, BC], dt.float32)
                    with tc.tile_pool(name="w3pool", bufs=3) as w3pool, \
                         tc.tile_pool(name="h3pool", bufs=18) as h3pool, \
                         tc.tile_pool(name="ps3", bufs=3, space="PSUM") as ps3:
                        h3_tiles = [None] * MT

                        def fc4_mms(m):
                            t_h3 = h3_tiles[m]
                            for n in range(NH):
                                nc.tensor.matmul(
                                    lg_psum[:, n * 512:(n + 1) * 512],
                                    w4_sb[:, m, :],
                                    t_h3[:, n * 512:(n + 1) * 512],
                                    start=(m == 0),
                                    stop=(m == MT - 1),
                                )

                        for m in range(MT):
                            if m == 0:
                                wsb = w3f
                            else:
                                wsb = w3pool.tile([128, KB, 2, 128],
                                                  dt.float8e4, tag="w3")
                                nc.sync.dma_start(out=wsb[:], in_=w3p[m])
                            psum = ps3.tile([128, BC], dt.float32, tag="ps3")
                            for n in range(NH):
                                for b in range(KB):
                                    nc.tensor.matmul(
                                        psum[:, n * 512:(n + 1) * 512],
                                        wsb[:, b],
                                        h2[:, 2 * b:2 * b + 2,
                                           n * 512:(n + 1) * 512],
                                        start=(b == 0),
                                        stop=(b == KB - 1),
                                        perf_mode=DR,
                                    )
                            t_h3 = h3pool.tile([128, BC], dt.float16, tag="h3")
                            nc.scalar.activation(t_h3[:], psum[:, :],
                                                 AF.Identity,
                                                 bias=b3_sb[:, m:m + 1])
                            nc.vector.tensor_scalar(t_h3[:], t_h3[:], 1.0, -1.0,
                                                    mybir.AluOpType.min,
                                                    mybir.AluOpType.max)
                            h3_tiles[m] = t_h3
                            # fc4 batched every 8 m-tiles (fewer stationary /
                            # perf-mode switches on the PE), pipelined one m
                            # behind so the PE never waits on this m's ACT/DVE
                            if m % 8 == 7 and m >= 15:
                                for mm in range(m - 15, m - 7):
                                    fc4_mms(mm)
                        for mm in range(MT - 8, MT):
                            fc4_mms(mm)

                    # ---------------- bias + log_softmax ----------------
                    lg_sb = lgsbp.tile([DOUT, BC], dt.float32)
                    nc.scalar.activation(lg_sb[:], lg_psum[:], AF.Identity,
                                         bias=b4_sb[:, 0:1])
                    NJ = BC // 128
                    with tc.tile_pool(name="tp", bufs=1, space="PSUM") as tpp, \
                         tc.tile_pool(name="sm", bufs=1) as smp:
                        # stage ops by function so the ACT engine loads each
                        # activation table once instead of per j-tile
                        shs = []
                        sums_all = smp.tile([128, NJ], dt.float32, tag="sums")
                        for j in range(NJ):
                            tp = tpp.tile([128, DOUT], dt.float32, tag=f"tp{j%4}")
                            nc.tensor.transpose(
                                tp[:], lg_sb[:, j * 128:(j + 1) * 128], ident[:])
                            rowmax = smp.tile([128, 1], dt.float32,
                                              tag=f"rmax{j}")
                            nc.vector.tensor_reduce(rowmax[:], tp[:],
                                                    mybir.AxisListType.X,
                                                    mybir.AluOpType.max)
                            sh = smp.tile([128, DOUT], dt.float32, tag=f"sh{j}")
                            nc.vector.tensor_scalar(sh[:], tp[:], rowmax[:],
                                                    None,
                                                    mybir.AluOpType.subtract)
                            shs.append(sh)
                        exs = smp.tile([128, DOUT], dt.float32, tag="exs")
                        for j in range(NJ):
                            nc.scalar.activation(exs[:], shs[j][:], AF.Exp,
                                                 accum_out=sums_all[:, j:j + 1])
                        lns_all = smp.tile([128, NJ], dt.float32, tag="lns")
                        nc.scalar.activation(lns_all[:], sums_all[:], AF.Ln)
                        for j in range(NJ):
                            res = smp.tile([128, DOUT], dt.float32,
                                           tag=f"res{j}")
                            nc.vector.tensor_scalar(res[:], shs[j][:],
                                                    lns_all[:, j:j + 1], None,
                                                    mybir.AluOpType.subtract)
                            nc.sync.dma_start(
                                out=out[j * 128:(j + 1) * 128, :], in_=res[:])

    nc.compile()
    return nc


def _pack_inputs(x, w1, b1, w2, b2, w3, b3, w4, b4):
    """Host-side packing into the device layouts. Shared tensors are packed
    once; only xt differs per core."""
    f32 = np.float32
    f16 = np.float16
    x = np.asarray(x, f32).reshape(B, DIN)

    # fc1 weights: sign(w1).T stacked twice (hi/lo terms share the weights),
    # padded to [1664, 6144], layout [q, p, k, m]
    s1 = np.sign(np.asarray(w1, f32))                       # [DH, DIN]
    s1t = np.zeros((K1P, DH), f16)
    s1t[:DIN] = s1.T
    s1t[DIN:2 * DIN] = s1.T
    w1t = np.ascontiguousarray(
        s1t.reshape(KT1, 128, MQ, MPQ * 128).transpose(2, 1, 0, 3))

    def pack_dr(w):
        # sign(w).T -> [mo, p, b, i, m'] DoubleRow stationary layout
        st = np.sign(np.asarray(w, f32)).T                  # [in, out]
        r = st.reshape(KB, 2, 128, MT, 128)                 # [b, i, p, mo, m']
        return np.ascontiguousarray(r.transpose(3, 2, 0, 1, 4)).astype(FP8)

    w2p = pack_dr(w2)
    w3p = pack_dr(w3)

    # fc4 weights: w4.T in fp16, layout [p, j, c]
    w4t = np.asarray(w4, f32).T.astype(f16)                 # [DH, DOUT]
    w4p = np.ascontiguousarray(w4t.reshape(MT, 128, DOUT).transpose(1, 0, 2))

    def pack_b(b):
        return np.ascontiguousarray(np.asarray(b, f32).reshape(MT, 128).T)

    b1p, b2p, b3p = pack_b(b1), pack_b(b2), pack_b(b3)
    b4p = np.asarray(b4, f32).reshape(DOUT, 1)

    shared = {"w1t": w1t, "w2p": w2p, "w3p": w3p, "w4p": w4p,
              "b1p": b1p, "b2p": b2p, "b3p": b3p, "b4p": b4p}

    # per-core x: fp16 hi/lo split stacked along contraction, layout [p, k, n]
    in_maps = []
    for c in range(CORES):
        xc = x[c * BC:(c + 1) * BC]                         # [BC, DIN]
        hi = xc.astype(f16)
        lo = (xc - hi.astype(f32)).astype(f16)
        arr = np.zeros((K1P, BC), f16)
        arr[:DIN] = hi.T
        arr[DIN:2 * DIN] = lo.T
        xt = np.ascontiguousarray(arr.reshape(KT1, 128, BC).transpose(1, 0, 2))
        in_maps.append({"xt": xt, **shared})
    return in_maps


_cached_nc = None


def kernel(x, w1, b1, w2, b2, w3, b3, w4, b4):
    global _cached_nc, last_exec_time_ns
    import os
    trace = bool(int(os.environ.get("KERNEL_TRACE", "0")))
    if _cached_nc is None:
        _cached_nc = _build_program()
    in_maps = _pack_inputs(x, w1, b1, w2, b2, w3, b3, w4, b4)
    res = run_bass_kernel_spmd(_cached_nc, in_maps, list(range(CORES)),
                               trace=trace)
    last_exec_time_ns = res.exec_time_ns
    return np.concatenate([res.results[c]["out"] for c in range(CORES)], axis=0)
